# revision 43
# baseline (speedup 1.0000x reference)
"""Trainium2 Bass kernel for nn_CrossAttentionModule.

Math insight: the query h3 is the masked-mean aspect vector h2_agg broadcast
over all S positions, so scores[b,h,q,k] do not depend on q.  The whole
[B,S,S] output is a single row row[b,k] broadcast along the q axis:

    qvec[b]   = Wq @ h2_agg[b]                      (H)
    v[b,j,:]  = Wk[j*hd:(j+1)*hd, :]^T @ qvec[b, j*hd:(j+1)*hd]   (per head)
    raw[b,j,s] = v[b,j,:] . h1[b,s,:]
    w_j = softmax_s(scale*raw_j + key_mask);  row[b,s] = mean_j w[b,j,s]
    out[b,q,s] = row[b,s]

Sharding (per the spec hint: tensor-parallel over num_heads, plus data
parallel over batch): 8 cores = 2 batches x 4 head-groups.  Core c owns
batch c//4 and heads 4*(c%4)..4*(c%4)+3.  Each core runs the COMPLETE
per-head masked softmax for its 4 heads over its batch (including the
per-head normalizer and the 1/NH factor) and stores the partial row
row_c[s] = sum_{j in core} w[j,s]/16 as f32.  The host gather does the
all-reduce-mean over the head shards (sums the 4 partial rows per batch)
and broadcasts the row along the q axis -- pure shard-assembly ops.

Per-core traffic is ~2.3 MB (vs ~7.1 MB unsharded): h1 for one batch as
fp8 (length-specialized to the valid 128-rounded prefix), 256-row slices
of Wq/Wk as fp8 packed into one load, h2 as bf16 (with the aspect-mask
column appended so the masked-sum matmuls consume it with zero prep
hops), the sentence mask, and a 7 KB f32 row store.  The serial
360 GB/s DMA pool is the binding resource; the stream is ordered
wP -> masks(SWDGE) -> h2 -> one h1 piece per softmax chunk, with chunk
widths [512, 512, 320, 320] balancing the last-piece tail (DMA-sem +
scores + exp) against the serial Act exp chain.

Quantization (mirrors the validated baseline scheme): h1*2 -> e4m3,
W*128 -> e3m4, device intermediates requantized with power-of-two
rescales so the net factor through the score matmul is 1.0; the exp
scale carries SCALE/aspect_len.  Scores accumulate in f32 PSUM;
DoubleRow fp8 matmuls score two 128-deep contraction chunks per
instruction (vt is zero-padded to 16 columns to satisfy the dual-fp8
Ldweights ISA restriction, and the extra rows are zeroed out of the
combine by lmat).

Tail structure: per-chunk exps write one stacked [128, 512] f16 tile
(chunk n at partition 32n -- engine partition offsets must be multiples
of 32) with the Act accumulator collecting Z, so the final
normalize+head-combine is a single matmul with a block-diagonal
[128, nch] lhsT carrying 1/(NH*Z_j), followed by one PSUM->SBUF copy
and one store DMA.  The additive key-mask rows are pre-loaded into each
chunk's PSUM tile right after the v chain (explicit no-sync deps keep
the in-order PE queue from reordering them ahead of it).
"""

import os
from contextlib import ExitStack

import ml_dtypes
import numpy as np

import concourse.bass as bass
import concourse.tile as tile
from concourse import bacc
from concourse import mybir

B, S, A, H = 2, 2048, 16, 1024
NH, HD = 16, 64
SCALE = float(HD) ** -0.5
NCORES = 8
NGRP = 4          # head groups (cores per batch)
HPG = NH // NGRP  # heads per group = 4
RPG = HPG * HD    # W rows per group = 256
NC_H = H // 128   # 8 contraction chunks
NEG = -1.0e30

F32 = mybir.dt.float32
F16 = mybir.dt.float16
BF16 = mybir.dt.bfloat16
F8 = mybir.dt.float8e3
F8E4 = mybir.dt.float8e4
U8 = mybir.dt.uint8
AF = mybir.ActivationFunctionType
DR = mybir.MatmulPerfMode.DoubleRow

# power-of-two quantization scales (see module docstring)
S_H1 = 2.0       # host: h1 * S_H1 -> e4m3
S_W = 128.0      # host: Wq*S_W, Wk*S_W -> e3m4
S_H2S = 0.125    # device: h2sum * S_H2S -> e3m4
S_QM = 0.5       # device: qm = qv_true * S_QM
S_VT = 0.5       # device: vt = v_true * S_VT  (S_VT * S_H1 == 1 -> scl unchanged)


TAIL_WS = [320, 320]  # trailing chunk widths appended after the 512s


def _chunk_widths(l_pad):
    """Softmax chunk widths covering [0, l_pad): 512s plus a tail split
    (small final chunks so little work trails the last h1 DMA).  Each
    chunk is one h1 DMA piece and one PSUM score tile (<= 512 f32)."""
    if l_pad > 1664:
        # keep nch <= 4: chunk n stacks at partition 32*n for the combine
        ws = [512, 512, 512, l_pad - 1536]
    else:
        tail = sum(TAIL_WS)
        ws = []
        rem = l_pad
        while rem > tail:
            w = min(512, rem - tail)
            ws.append(w)
            rem -= w
        for w in TAIL_WS:
            if rem <= 0:
                break
            w = min(w, rem)
            ws.append(w)
            rem -= w
    assert sum(ws) == l_pad and all(w <= 512 for w in ws) and len(ws) <= 4
    return ws


def _build_kernel(l_pad, mask_lo):
    """One SPMD program: batch/head-group selection happens purely via
    the per-core input data.  mask_lo: first column from which the
    additive key mask is applied (0 = mask everything)."""
    widths = _chunk_widths(l_pad)
    nch = len(widths)
    gcols = [sum(widths[:n]) for n in range(nch)]
    assert mask_lo in gcols, (mask_lo, gcols)

    nc = bacc.Bacc("TRN2")
    h1P_d = nc.dram_tensor("h1P", [1, H * l_pad], F8E4, kind="ExternalInput")
    h2_d = nc.dram_tensor("h2", [A, H + 1], BF16, kind="ExternalInput")
    masks_d = nc.dram_tensor("masks", [1, S], U8, kind="ExternalInput")
    # Wq^T slice and Wk slice packed into one [128, 4096] fp8 load
    wp_d = nc.dram_tensor("wP", [128, NC_H * RPG + (RPG // 128) * H], F8,
                          kind="ExternalInput")
    out_d = nc.dram_tensor("out", [1, nch * 512], F32, kind="ExternalOutput")

    from concourse.tile_rust import add_dep_helper

    with tile.TileContext(nc) as tc, ExitStack() as ctx:
        consts = ctx.enter_context(tc.tile_pool(name="consts", bufs=1))
        small = ctx.enter_context(tc.tile_pool(name="small", bufs=2))
        wqp = ctx.enter_context(tc.tile_pool(name="wqp", bufs=1))
        h1tp = ctx.enter_context(tc.tile_pool(name="h1tp", bufs=1))
        wsp = ctx.enter_context(tc.tile_pool(name="wsp", bufs=1))
        obp = ctx.enter_context(tc.tile_pool(name="obp", bufs=1))
        pss = ctx.enter_context(tc.tile_pool(name="pss", bufs=1, space="PSUM"))
        psv = ctx.enter_context(tc.tile_pool(name="psv", bufs=1, space="PSUM"))
        psc = ctx.enter_context(tc.tile_pool(name="psc", bufs=4, space="PSUM"))
        psb = ctx.enter_context(tc.tile_pool(name="psb", bufs=1, space="PSUM"))

        ones128 = consts.tile([1, 128], F32, tag="ones128")
        nc.vector.memset(ones128, 1.0)
        ones16r = consts.tile([1, 16], BF16, tag="ones16r")
        nc.vector.memset(ones16r, 1.0)
        # lmat[32n+j, n] = 1/(NH * Z_j): zeroed early, filled at the tail
        # (chunk blocks sit at partition 32n -- engine partition offsets
        # must be multiples of 32)
        lmat = consts.tile([128, nch], F16, tag="lmat")
        nc.vector.memset(lmat, 0.0)

        # Exp act-table preload, long before the first real exp
        dume = small.tile([1, 16], F32, tag="dume")
        nc.scalar.activation(dume, ones128[:, 0:16], AF.Exp)

        # ---- the DMA stream ----
        # tiny loads ride the Pool/SWDGE queue (keeps them off the serial
        # HWDGE device); big loads ride SP, one h1 piece per score chunk
        wp = wqp.tile([128, NC_H * RPG + (RPG // 128) * H], F8, tag="wp")
        i_wp = nc.sync.dma_start(wp, wp_d[:, :])
        mask_sb = small.tile([1, S], U8, tag="mask_sb")
        i_mask = nc.gpsimd.dma_start(mask_sb, masks_d[:, :])
        h2t = small.tile([A, H + 1], BF16, tag="h2t")
        i_h2 = nc.sync.dma_start(h2t, h2_d[:, :])
        wqT = wp[:, 0:NC_H * RPG].rearrange("p (c r) -> p c r", c=NC_H)
        wk = wp[:, NC_H * RPG:].rearrange("p (c h) -> p c h", c=RPG // 128)
        h1t = []
        h1_insts = []
        off = 0
        for pi, pw in enumerate(widths):
            t = h1tp.tile([128, NC_H, pw], F8E4, tag=f"h1t_{pi}",
                          name=f"h1t_{pi}")
            h1_insts.append(nc.sync.dma_start(
                t.rearrange("p c w -> p (c w)"),
                h1P_d[0, off:off + H * pw].rearrange("(p x) -> p x", p=128)))
            h1t.append(t)
            off += H * pw
        chain = [i_wp, i_h2] + h1_insts
        for i in range(1, len(chain)):
            add_dep_helper(chain[i].ins, chain[i - 1].ins,
                           sync=False, reason="dma stream order")

        # ---- aspect prep: the aspect-mask column rides as h2's last
        # column, so the masked-sum matmuls consume it with zero hops ----
        am_col = h2t[:, H:H + 1]

        # ---- h2sumT[p, c] = sum_a m[a] h2[a, c*128+p]  (unscaled) ----
        h2sT_ps = pss.tile([128, NC_H, 1], F32, tag="pssmall", name="h2sT_ps")
        for c in range(NC_H):
            nc.tensor.matmul(
                h2sT_ps[:, c, :],
                lhsT=h2t[:, c * 128:(c + 1) * 128],
                rhs=am_col,
            )
        h2sT = small.tile([128, NC_H, 1], F8, tag="h2sT")
        nc.vector.tensor_scalar_mul(h2sT, h2sT_ps, S_H2S)

        # additive key mask row over [mask_lo, l_pad): 0 valid / -1e30 masked
        mw = l_pad - mask_lo
        mb = small.tile([1, mw], BF16, tag="mb")
        nc.scalar.activation(mb, mask_sb[0:1, mask_lo:l_pad], AF.Copy,
                             bias=NEG, scale=-NEG)

        # ---- qvec' for this core's 4 heads (256 rows of Wq) ----
        qv_ps = pss.tile([128, RPG // 128, 1], F32, tag="pssmall",
                         name="qv_ps")
        for m in range(RPG // 128):
            for c in range(NC_H):
                nc.tensor.matmul(
                    qv_ps[:, m, :],
                    lhsT=wqT[:, c, m * 128:(m + 1) * 128],
                    rhs=h2sT[:, c, :],
                    start=(c == 0),
                    stop=(c == NC_H - 1),
                )

        # ---- masked per-head qvec columns straight from PSUM: qm[d, c, jl]
        # = qvec[d] iff d in head jl's 64-row block (jl = 2*c + (d >= 64)) ----
        qm_scale = S_QM / (S_W * S_H2S)
        qm = small.tile([128, RPG // 128, HPG], F8, tag="qm")
        nc.vector.memset(qm, 0.0)
        for m in range(RPG // 128):
            nc.vector.tensor_scalar_mul(
                qm[0:64, m, 2 * m:2 * m + 1], qv_ps[0:64, m, :], qm_scale)
            nc.vector.tensor_scalar_mul(
                qm[64:128, m, 2 * m + 1:2 * m + 2], qv_ps[64:128, m, :],
                qm_scale)

        # ---- vT[i-part, i-chunk, jl] = Wk_rows^T @ qm ----
        vt_ps = psv.tile([128, NC_H, HPG], F32, tag="psvt", name="vt_ps")
        for m in range(NC_H):
            for c in range(RPG // 128):
                nc.tensor.matmul(
                    vt_ps[:, m, :],
                    lhsT=wk[:, c, m * 128:(m + 1) * 128],
                    rhs=qm[:, c, :],
                    start=(c == 0),
                    stop=(c == RPG // 128 - 1),
                )
        # vt padded to 16 columns (zeros beyond the 4 real heads): the
        # dual-row fp8 Ldweights requires the baseline's 16-wide layout
        vt = small.tile([128, NC_H, 16], F8E4, tag="vt")
        nc.vector.memset(vt, 0.0)
        i_vtmul = nc.vector.tensor_scalar_mul(
            vt[:, :, 0:HPG], vt_ps, S_VT / (S_W * S_QM))

        # exp scale = SCALE / aspect_len (runs parallel to the vt chain)
        ones16c = consts.tile([A, 1], BF16, tag="ones16c")
        nc.vector.memset(ones16c, 1.0)
        alen_ps = pss.tile([1, 1], F32, tag="pssmall", name="alen_ps")
        nc.tensor.matmul(alen_ps, lhsT=am_col, rhs=ones16c)
        alen = small.tile([1, 1], F32, tag="alen")
        nc.vector.tensor_scalar_max(alen, alen_ps, 1.0)
        rlen = small.tile([1, 1], F32, tag="rlen")
        nc.vector.reciprocal(rlen, alen)
        r16_ps = pss.tile([16, 1], F32, tag="pssmall", name="r16_ps")
        nc.tensor.matmul(r16_ps, lhsT=ones128[:, 0:16], rhs=rlen)
        scl = small.tile([16, 1], F32, tag="scl")
        nc.vector.tensor_scalar_mul(scl, r16_ps, SCALE)


        # ---- scores + exp per chunk; exps stack into one [4*nch, 512]
        # tile (chunk n at partitions 4n..4n+3) for the one-shot combine ----
        wstack = wsp.tile([128, 512], F16, tag="wstack")
        nc.vector.memset(wstack, 0.0)
        zbuf = small.tile([16, nch], F32, tag="zbuf")
        nc.vector.memset(zbuf, 0.0)
        z2t = small.tile([HPG, 1], F32, tag="z2t")
        # pre-hoist the additive key-mask rows into each chunk's PSUM tile
        # (no h1 dependency; placed after the vt chain so the in-order PE
        # queue keeps the prep chain first)
        scs, maskedv = [], []
        for n in range(nch):
            cw, gcol = widths[n], gcols[n]
            masked = gcol + cw > mask_lo
            sc = psc.tile([16, cw], F32, tag="sc", name=f"sc_{n}")
            scs.append(sc)
            maskedv.append(masked)
            if masked:
                i_mm = nc.tensor.matmul(
                    sc, lhsT=ones16r,
                    rhs=mb[:, gcol - mask_lo:gcol - mask_lo + cw],
                    start=True, stop=False,
                )
                add_dep_helper(i_mm.ins, i_vtmul.ins, sync=False,
                               reason="keep prep chain first on PE")
        for n in range(nch):
            cw, gcol = widths[n], gcols[n]
            sc, masked = scs[n], maskedv[n]
            for m2 in range(NC_H // 2):
                nc.tensor.matmul(
                    sc,
                    lhsT=vt[:, 2 * m2:2 * m2 + 2, :],
                    rhs=h1t[n][:, 2 * m2:2 * m2 + 2, 0:cw],
                    start=(not masked and m2 == 0),
                    stop=(m2 == NC_H // 2 - 1),
                    perf_mode=DR,
                )
            if n == nch - 2:
                # keep the e3-gating Act aux off the chain: chunk nch-2's Z
                # rides the idle DVE instead (its reduce finishes before
                # the final chunk's accumulator read)
                nc.scalar.activation(
                    wstack[32 * n:32 * n + 16, 0:cw], sc, AF.Exp,
                    bias=0.0, scale=scl)
                nc.vector.reduce_sum(
                    z2t, wstack[32 * n:32 * n + HPG, 0:cw],
                    axis=mybir.AxisListType.X)
            else:
                nc.scalar.activation(
                    wstack[32 * n:32 * n + 16, 0:cw], sc, AF.Exp,
                    bias=0.0, scale=scl, accum_out=zbuf[:, n:n + 1])

        # ---- normalizer: lmat[4n+j, n] = 1 / (NH * Z_j) ----
        ztot = small.tile([HPG, 1], F32, tag="ztot")
        nc.vector.reduce_sum(ztot, zbuf[0:HPG, :], axis=mybir.AxisListType.X)
        nc.vector.tensor_tensor(ztot, ztot, z2t, mybir.AluOpType.add)
        rz = small.tile([HPG, 1], F32, tag="rz")
        nc.vector.reciprocal(rz, ztot)
        for n in range(nch):
            nc.vector.tensor_scalar_mul(
                lmat[32 * n:32 * n + HPG, n:n + 1], rz, 1.0 / NH)

        # ---- one-shot head-combine + normalize: bc[n, s] = partial row ----
        bc = psb.tile([nch, 512], F32, tag="bc")
        nc.tensor.matmul(bc, lhsT=lmat, rhs=wstack)
        ob = obp.tile([nch, 512], F32, tag="ob")
        nc.vector.tensor_copy(ob, bc)
        nc.sync.dma_start(
            out_d[0, :].rearrange("(p c) -> p c", p=nch), ob)

    nc.finalize()
    return nc


_NC_CACHE = {}


def kernel(h1, h2, sentence_mask, aspect_mask, Wq, Wk):
    from concourse.bass_utils import run_bass_kernel_spmd

    sm = np.ascontiguousarray(sentence_mask).astype(bool)
    am = np.ascontiguousarray(aspect_mask).astype(bool)
    lens_true = sm.sum(axis=1)
    prefix_ok = all(
        sm[b, :lens_true[b]].all() and not sm[b, lens_true[b]:].any()
        for b in range(B))
    if prefix_ok and all(int(l) >= 512 for l in lens_true):
        l_pad = int(max(min(S, -(-int(l) // 128) * 128) for l in lens_true))
        mask_lo = int(min(lens_true))
        # mask chunks only from the first chunk that can contain a masked
        # column; snap mask_lo to the chunk grid
        widths = _chunk_widths(l_pad)
        gcols = [sum(widths[:n]) for n in range(len(widths))]
        mask_lo = max(g for g in gcols if g <= mask_lo)
    else:
        l_pad, mask_lo = S, 0

    key = (l_pad, mask_lo)
    if key not in _NC_CACHE:
        _NC_CACHE[key] = _build_kernel(l_pad, mask_lo)
    nc = _NC_CACHE[key]
    kernel.last_nc = nc

    f8 = ml_dtypes.float8_e3m4
    widths = _chunk_widths(l_pad)
    gcols = [sum(widths[:n]) for n in range(len(widths))]

    # ---- host staging: shard + quantize + lay out in SBUF order ----
    wq_q = np.clip(np.asarray(Wq, np.float32) * S_W, -15.5, 15.5).astype(f8)
    wk_q = np.clip(np.asarray(Wk, np.float32) * S_W, -15.5, 15.5).astype(f8)
    h2_bf = np.ascontiguousarray(np.asarray(h2)).astype(ml_dtypes.bfloat16)
    h1_q = np.clip(np.asarray(h1, np.float32) * S_H1, -240.0, 240.0) \
        .astype(ml_dtypes.float8_e4m3)

    in_maps = []
    h1flat_b = {}
    for b in range(B):
        # h1[b] transposed to [H, l_pad], fp8, staged piece-contiguously:
        # each piece is a [128, NC_H, w] block contiguous per partition row
        h1q = h1_q[b].T[:, :l_pad].reshape(NC_H, 128, l_pad)
        h1flat = np.empty(H * l_pad, ml_dtypes.float8_e4m3)
        off = oe = 0
        for pw in widths:
            h1flat[oe:oe + H * pw] = np.ascontiguousarray(
                h1q[:, :, off:off + pw].transpose(1, 0, 2)).reshape(-1)
            off += pw
            oe += H * pw
        h1flat_b[b] = h1flat.reshape(1, H * l_pad)
    for core in range(NCORES):
        b, g = core // NGRP, core % NGRP
        wqTP = np.ascontiguousarray(
            wq_q[g * RPG:(g + 1) * RPG, :].T).reshape(NC_H, 128, RPG) \
            .transpose(1, 0, 2).reshape(128, NC_H * RPG)
        wkP = wk_q[g * RPG:(g + 1) * RPG, :].reshape(RPG // 128, 128, H) \
            .transpose(1, 0, 2).reshape(128, (RPG // 128) * H)
        in_maps.append({
            "h1P": h1flat_b[b],
            "h2": np.ascontiguousarray(np.concatenate(
                [h2_bf[b], am[b].astype(ml_dtypes.bfloat16)[:, None]],
                axis=1)),
            "masks": sm[b].view(np.uint8).reshape(1, S),
            "wP": np.ascontiguousarray(
                np.concatenate([wqTP, wkP], axis=1)),
        })

    trace = bool(int(os.environ.get("KERNEL_TRACE", "0")))
    res = run_bass_kernel_spmd(
        nc,
        in_maps,
        core_ids=list(range(NCORES)),
        trace=trace,
    )
    if trace and res.exec_time_ns is not None:
        kernel.last_exec_time_ns = res.exec_time_ns
        kernel.last_results = res

    # ---- gather: all-reduce-mean over head shards, broadcast over q ----
    rows = np.zeros((B, S), np.float32)
    for core in range(NCORES):
        b = core // NGRP
        obuf = np.asarray(res.results[core]["out"], np.float32).reshape(-1)
        for n, (cw, gcol) in enumerate(zip(widths, gcols)):
            rows[b, gcol:gcol + cw] += obuf[n * 512:n * 512 + cw]
    out = np.empty((B, S, S), np.float32)
    out[:] = rows[:, None, :]
    return out
